# revision 30
# baseline (speedup 1.0000x reference)
"""Bahdanau-style attention scores kernel for Trainium2 (8 NeuronCores).

Reference computation (B=32, S=2048, ENC_H=512, DEC_H=1024):
    W_s = attn_w[:, :1024]; W_e = attn_w[:, 1024:]
    proj_s = s @ W_s.T                      # [B, 1024]
    proj_e = enc @ W_e.T                    # [B, S, 1024]
    scores = tanh(proj_s[:, None] + proj_e) @ v_w.T   # [B, S]
    out = softmax(scores, axis=1)

Strategy: data-parallel over batch (4 batches per core). Everything is
core-local, including the softmax, so there are no collectives.

On-device layout: all matmuls keep the hidden dim h on PSUM partitions:
    projT[h, s] = sum_e W_eT[e, h] * encT[e, s]
so the per-batch proj_s bias is a per-partition scalar (fused into the
ACT tanh) and the v-dot runs as 2 rounds of 4 concurrent col-tiled
matmuls (M=32 with v replicated across 32 columns so every PSUM
partition is written), followed by a 1/32-weighted reduce matmul.

Prologue engineering: a short stream of dummy matmuls keeps the PE HAM
activity monitor busy from ~7us so real matmuls run at 2.4 GHz instead
of the cold 1.2 GHz; weights arrive in h-quarter chunks (hc-major) so
the first matmul/tanh groups gate on ~512KB instead of 2MB; the first
batch's encoder stream lands in small leading pieces. proj_s matmuls
are interleaved with the first piece's main groups so neither blocks
the other. The host passes encoder_outputs pre-transposed to [b, E, S]
(pure layout change, f32); f32 -> bf16 conversion of the enc stream
happens inside the SWDGE DMA (cast-on-load). The small replicated
weights are pre-cast to bf16 on the host and loaded via HWDGE.
"""

import numpy as np
import ml_dtypes

import concourse.bass as bass
import concourse.tile as tile
from concourse import mybir
from concourse.bass_utils import run_bass_kernel_spmd

N_CORES = 8
B, S = 32, 2048
E = 1024  # 2*ENC_H, contraction dim of the big matmul
H = 1024  # DEC_H, hidden dim of tanh
D = 1024  # DEC_H, contraction dim of proj_s
BPC = B // N_CORES  # batches per core
P = 128
EC, HC, DC = E // P, H // P, D // P

# s-piece schedule: uniform 512 pieces; the last batch ends small so the
# softmax tail chain after the final matmul is short.
PIECES_B0 = [512] * 4
PIECES = [512] * 4
PIECES_LAST = [512, 512, 512, 384, 128]
N_DUMMY = 270  # HAM warm-up matmuls (N=8 each, ~25ns apiece)

F32 = mybir.dt.float32
BF16 = mybir.dt.bfloat16
NP_BF16 = ml_dtypes.bfloat16

_cache = {}


def _split_multiwaits(nc):
    """Walrus in this toolchain rejects instructions carrying more than one
    semaphore wait ("Too many sync wait commands"). Engine queues dispatch in
    order, so moving the extra waits onto same-engine NoOps just before the
    instruction is semantically identical."""
    for fn in nc.m.functions:
        for blk in fn.blocks:
            out = []
            for inst in blk.instructions:
                si = inst.sync_info
                waits = list(si.on_wait) if si is not None and si.on_wait else []
                if len(waits) > 1:
                    for i, w in enumerate(waits[:-1]):
                        out.append(
                            mybir.InstNoOp(
                                name=f"{inst.name}-w{i}",
                                engine=inst.engine,
                                sync_info=mybir.SyncInfo(on_wait=[w], on_update=[]),
                                bass_nofuse=True,
                            )
                        )
                    si.on_wait = [waits[-1]]
                    inst.sync_info = si
                out.append(inst)
            try:
                blk.instructions = out
            except Exception:
                blk.set_instructions(out)


def _dedup_ldweights(nc):
    """Tile lowers every matmul to an Ldweights/Matmult pair. When consecutive
    matmuls use the same stationary weights (the dummy warm-up stream), the
    second Ldweights reloads identical array state — drop it and carry its
    waits over to the next PE instruction (split later by _split_multiwaits)."""
    ndrop = 0
    for fn in nc.m.functions:
        for blk in fn.blocks:
            out = []
            loaded = None
            pending_waits = []
            for inst in blk.instructions:
                if getattr(inst, "engine", None) != mybir.EngineType.PE:
                    out.append(inst)
                    continue
                if pending_waits:
                    si = inst.sync_info or mybir.SyncInfo(on_wait=[], on_update=[])
                    si.on_wait = list(si.on_wait) + pending_waits
                    inst.sync_info = si
                    pending_waits = []
                if isinstance(inst, mybir.InstLdweights):
                    ap = inst.ins[0]
                    key = (
                        ap.memref,
                        ap.offset,
                        str(ap.ap),
                        str(ap.dtype),
                        str(getattr(inst, "tile_position", None)),
                    )
                    if key == loaded:
                        si = inst.sync_info
                        if si is not None and si.on_wait:
                            pending_waits = list(si.on_wait)
                        if si is not None and si.on_update:
                            # keep the instruction if someone depends on it
                            out.append(inst)
                            continue
                        ndrop += 1
                        continue
                    loaded = key
                elif isinstance(inst, mybir.InstMatmult):
                    pass  # matmuls stream against loaded weights
                else:
                    loaded = None  # unknown PE instruction: be conservative
                out.append(inst)
            assert not pending_waits
            try:
                blk.instructions = out
            except Exception:
                blk.set_instructions(out)
    return ndrop


def _pieces(b):
    if b == 0:
        sched = PIECES_B0
    elif b == BPC - 1:
        sched = PIECES_LAST
    else:
        sched = PIECES
    off = 0
    out = []
    for w in sched:
        out.append((off, w))
        off += w
    return out


def _build_bass():
    nc = bass.Bass()
    # All weight tensors arrive pre-swizzled into partition-major slabs so
    # every DMA is 128 descriptors of >=4KB (descriptor count, not bytes,
    # is what throttles the DGE rings).
    enc_t = nc.dram_tensor("enc_t", [BPC, E, S], BF16, kind="ExternalInput")
    w_q = nc.dram_tensor("w_q", [4, P, 2 * EC * P], BF16, kind="ExternalInput")
    ws_q = nc.dram_tensor("ws_q", [4, P, 2 * DC * P], BF16, kind="ExternalInput")
    s_p = nc.dram_tensor("s_p", [P, DC * BPC], BF16, kind="ExternalInput")
    vr_p = nc.dram_tensor("vr_p", [P, HC * 32 + 1], BF16, kind="ExternalInput")
    out = nc.dram_tensor("out", [BPC, S], F32, kind="ExternalOutput")

    Tanh = mybir.ActivationFunctionType.Tanh
    Exp = mybir.ActivationFunctionType.Exp

    with tile.TileContext(nc) as tc:
        with (
            tc.tile_pool(name="consts", bufs=1) as consts,
            tc.tile_pool(name="enc", bufs=3) as enc_pool,
            tc.tile_pool(name="tanh", bufs=10) as tanh_pool,
            tc.tile_pool(name="scc", bufs=2) as scc_pool,
            tc.tile_pool(name="rows", bufs=2) as row_pool,
            tc.tile_pool(name="mmps", bufs=4, space="PSUM") as mm_psum,
            tc.tile_pool(name="scps", bufs=2, space="PSUM") as sc_psum,
            tc.tile_pool(name="psps", bufs=2, space="PSUM") as ps_psum,
        ):
            # HAM warm-up: PE busy from the end of the framework preamble so
            # the clock gate opens (1.2 -> 2.4 GHz) before real work arrives.
            # The dummy tile is memset (not DMA'd) so nothing gates it.
            dummy = consts.tile([P, 8], BF16)
            nc.vector.memset(dummy, 0.0)
            dps = ps_psum.tile([1, 8], F32, tag="psps")
            for _ in range(N_DUMMY):
                nc.tensor.matmul(dps, dummy[:, 0:1], dummy, start=True, stop=True)

            # Weights arrive in hc-pair quarters (4KB contiguous run per
            # partition, 128 descriptors each), interleaved across the two
            # HWDGE rings in consumption order (projs needs ws-hc_k just
            # before main needs w-hc_k), so sustained PE work starts as soon
            # as the first quarter lands and never stalls on a later one.
            w_sb = consts.tile([P, HC, EC, P], BF16)
            ws_sb = consts.tile([P, HC, DC, P], BF16)
            s_sb = consts.tile([P, DC, BPC], BF16)
            nc.scalar.dma_start(
                out=s_sb[:], in_=s_p[:].rearrange("p (dc b) -> p dc b", dc=DC)
            )
            vr_sb = consts.tile([P, HC * 32 + 1], BF16)
            nc.scalar.dma_start(out=vr_sb[:], in_=vr_p[:])
            red_sb = vr_sb[:, HC * 32 : HC * 32 + 1]

            def wq_dma(eng, sb, src, q):
                eng.dma_start(
                    out=sb[:, 2 * q : 2 * q + 2],
                    in_=src[q].rearrange("p (h ec c) -> p h ec c", h=2, ec=EC),
                )

            wq_dma(nc.sync, w_sb, w_q, 0)
            wq_dma(nc.scalar, ws_sb, ws_q, 0)
            wq_dma(nc.sync, ws_sb, ws_q, 1)
            wq_dma(nc.scalar, w_sb, w_q, 1)
            wq_dma(nc.sync, w_sb, w_q, 2)
            wq_dma(nc.scalar, ws_sb, ws_q, 2)
            wq_dma(nc.sync, ws_sb, ws_q, 3)
            wq_dma(nc.scalar, w_sb, w_q, 3)

            def w_tile(sb, ec, hc):
                return sb[:, hc, ec, :]

            projs_sb = consts.tile([P, HC, BPC], F32)

            def projs_chunk(hc):
                # projsT[h, b] = sum_d W_sT[d, h] * sT[d, b] for one h-chunk
                pp = ps_psum.tile([P, BPC], F32, tag="psps")
                for dc in range(DC):
                    nc.tensor.matmul(
                        pp,
                        w_tile(ws_sb, dc, hc),
                        s_sb[:, dc, :],
                        start=(dc == 0),
                        stop=(dc == DC - 1),
                    )
                nc.vector.tensor_copy(projs_sb[:, hc, :], pp)

            # Flat pipeline over all (batch, piece) jobs. Per piece p the PE
            # stream is: [mm groups hc=0..7 of p] ... with p's v-rounds and
            # the reduce-mm of p-1 emitted after the FIRST mm group of p+1,
            # so the v-rounds never wait on p's last tanh and the reduce-mm
            # never waits on the DVE psum->sbuf copy.
            enc_tiles = {}
            for b in range(BPC):
                encT = enc_pool.tile([P, EC, S], BF16, tag="enc", name=f"encT{b}")
                enc_view = enc_t[b].rearrange("(ec p) s -> p ec s", p=P)
                if b == 0:
                    for off, w in _pieces(b):
                        sl = slice(off, off + w)
                        nc.gpsimd.dma_start(
                            out=encT[:, :, sl], in_=enc_view[:, :, sl]
                        )
                else:
                    for half in range(2):
                        sl = slice(half * 1024, (half + 1) * 1024)
                        nc.gpsimd.dma_start(
                            out=encT[:, :, sl], in_=enc_view[:, :, sl]
                        )
                enc_tiles[b] = encT

            rows = {}  # b -> (exp_row, sums)
            jobs = []
            for b in range(BPC):
                for pi, (off, w) in enumerate(_pieces(b)):
                    jobs.append((b, pi, off, w))

            state = {"v": None, "red": None}

            def emit_reduce():
                # reduce-mm + exp of the piece whose psum->sbuf copy is done
                if state["red"] is None:
                    return
                b, pi, off, w, scc = state["red"]
                state["red"] = None
                exp_row, sums = rows[b]
                sc2 = ps_psum.tile([1, 512], F32, tag="psps")
                nc.tensor.matmul(
                    sc2[:, :w], red_sb, scc[:, :w], start=True, stop=True
                )
                nc.scalar.activation(
                    exp_row[:, off : off + w],
                    sc2[:, :w],
                    Exp,
                    accum_out=sums[:, pi : pi + 1],
                )
                if pi == len(_pieces(b)) - 1:
                    npc = len(_pieces(b))
                    tot = row_pool.tile([1, 1], F32, tag="tot")
                    nc.vector.reduce_sum(
                        tot, sums[:, :npc], axis=mybir.AxisListType.X
                    )
                    rtot = row_pool.tile([1, 1], F32, tag="rtot")
                    nc.vector.reciprocal(rtot, tot)
                    out_row = row_pool.tile([1, S], F32, tag="out_row")
                    # halves, so the first out-DMA overlaps the second scale
                    hs = S // 2
                    for hh in range(2):
                        sl2 = slice(hh * hs, (hh + 1) * hs)
                        nc.vector.tensor_scalar_mul(
                            out_row[:, sl2], exp_row[:, sl2], rtot
                        )
                        nc.sync.dma_start(
                            out=out[b : b + 1, sl2], in_=out_row[:, sl2]
                        )

            def emit_v():
                # v-dot of the piece whose tanh tiles are all complete:
                # 2 rounds x 4 concurrent col-tiled matmuls. v is replicated
                # over 32 columns so all 128 PSUM partitions hold valid
                # partials (no garbage for the reduce matmul).
                if state["v"] is None:
                    return
                b, pi, off, w, ths = state["v"]
                state["v"] = None
                sc_ps = sc_psum.tile([P, 512], F32, tag="scps")
                for r in range(2):
                    for j in range(4):
                        hc = r * 4 + j
                        nc.tensor.matmul(
                            sc_ps[32 * j : 32 * (j + 1), :w],
                            vr_sb[:, hc * 32 : (hc + 1) * 32],
                            ths[hc][:, :w],
                            start=(r == 0),
                            stop=(r == 1),
                            tile_position=(0, 32 * j),
                        )
                emit_reduce()
                scc = scc_pool.tile([P, 512], BF16, tag="scc")
                nc.vector.tensor_copy(scc[:, :w], sc_ps[:, :w])
                state["red"] = (b, pi, off, w, scc)

            for b, pi, off, w in jobs:
                if pi == 0:
                    exp_row = row_pool.tile([1, S], F32, tag="exp_row")
                    sums = row_pool.tile(
                        [1, max(len(PIECES_B0), len(PIECES_LAST))],
                        F32,
                        tag="sums",
                    )
                    rows[b] = (exp_row, sums)
                encT = enc_tiles[b]
                sl = slice(off, off + w)
                ths = []
                for hc in range(HC):
                    if b == 0 and pi == 0:
                        projs_chunk(hc)
                    mm_ps = mm_psum.tile([P, 512], F32, tag="mmps")
                    for ec in range(EC):
                        nc.tensor.matmul(
                            mm_ps[:, :w],
                            w_tile(w_sb, ec, hc),
                            encT[:, ec, sl],
                            start=(ec == 0),
                            stop=(ec == EC - 1),
                        )
                    th = tanh_pool.tile([P, 512], BF16, tag="tanh")
                    nc.scalar.activation(
                        th[:, :w], mm_ps[:, :w], Tanh,
                        bias=projs_sb[:, hc, b : b + 1],
                    )
                    ths.append(th)
                    if hc == 0:
                        emit_v()
                state["v"] = (b, pi, off, w, ths)
            emit_v()
            emit_reduce()

    _dedup_ldweights(nc)
    _split_multiwaits(nc)
    return nc


def _prep_inputs(s, encoder_outputs, attn_w, v_w):
    s = np.asarray(s, dtype=np.float32)
    enc = np.asarray(encoder_outputs, dtype=np.float32)
    attn_w = np.asarray(attn_w, dtype=np.float32)
    v_w = np.asarray(v_w, dtype=np.float32)

    def hc_slab(w_t):
        # [X, H] -> [4, P, 2*XC*P] partition-major hc-pair quarter slabs
        xc = w_t.shape[0] // P
        tmp = w_t.astype(NP_BF16).reshape(xc, P, 4, 2, P)
        return np.ascontiguousarray(
            tmp.transpose(2, 1, 3, 0, 4).reshape(4, P, 2 * xc * P)
        )

    w_q = hc_slab(attn_w[:, D:].T)  # from [E, H]
    ws_q = hc_slab(attn_w[:, :D].T)  # from [D, H]
    # [P, HC*32+1]: v replicated over 32 cols per h-chunk, then the 1/32
    # reduction weight in the last column
    v_t = v_w.reshape(HC, P).T.astype(NP_BF16)  # [P, HC]
    vr_p = np.concatenate(
        [
            np.repeat(v_t, 32, axis=1),
            np.full((P, 1), 1.0 / 32.0, dtype=NP_BF16),
        ],
        axis=1,
    )
    vr_p = np.ascontiguousarray(vr_p)

    in_maps = []
    for c in range(N_CORES):
        lo, hi = c * BPC, (c + 1) * BPC
        # [BPC, E, S] bf16: pre-cast on host so the device DMA reads half
        # the HBM bytes (the f32->bf16 cast-on-load path reads f32)
        enc_t = np.ascontiguousarray(
            enc[lo:hi].astype(NP_BF16).transpose(0, 2, 1)
        )
        # [P, DC*BPC] partition-major packing of s^T
        s_p = np.ascontiguousarray(
            s[lo:hi].T.astype(NP_BF16).reshape(DC, P, BPC)
            .transpose(1, 0, 2).reshape(P, DC * BPC)
        )
        in_maps.append(
            {"enc_t": enc_t, "w_q": w_q, "ws_q": ws_q, "s_p": s_p, "vr_p": vr_p}
        )
    return in_maps


def _run(s, encoder_outputs, attn_w, v_w, trace=False):
    if "nc" not in _cache:
        _cache["nc"] = _build_bass()
    nc = _cache["nc"]
    in_maps = _prep_inputs(s, encoder_outputs, attn_w, v_w)
    res = run_bass_kernel_spmd(nc, in_maps, list(range(N_CORES)), trace=trace)
    out = np.concatenate([res.results[c]["out"] for c in range(N_CORES)], axis=0)
    return out.astype(np.float32), res


def kernel(s, encoder_outputs, attn_w, v_w):
    out, _ = _run(s, encoder_outputs, attn_w, v_w, trace=False)
    return out
